# revision 3
# baseline (speedup 1.0000x reference)
"""HME (hierarchical mixture of experts) kernel for 8 Trainium2 NeuronCores.

Strategy: expert-parallel over the 64 leaves (8 leaves per core).
Each core:
  - computes the full gating network (replicated, tiny):
      z = x_gating @ gw + gb  (fp32 matmuls)
      log leaf prob = sum over path nodes of -softplus(-+z)  (path matmul)
      lp = exp(...)
  - computes partial[b, o] = sum_{its 8 leaves} lp[b,l] * (x_leaf @ pw[:,:,l].T + pb[:,l])
      main matmuls in float32r (full PE speed, ~1.5e-4 matmul error)
      per-leaf PSUM drain scaled by lp via DVE scalar_tensor_tensor
  - ReduceScatter(add) over the 8 cores -> each core owns 128 batch rows
Host: concatenates the 8 row-slices.
"""
import os
import sys

sys.path.insert(0, '/opt/trn_rl_repo')

import numpy as np
import concourse.bass as bass
import concourse.bacc as bacc
import concourse.tile as tile
from concourse import mybir
from concourse.bass_utils import run_bass_kernel_spmd

B = 1024
GF = 512          # gating features
IF = 512          # in features
OF = 512          # out features
L = 64            # leaves
G = 63            # internal gate nodes
DEPTH = 6
NCORES = 8
LPC = L // NCORES # leaves per core
NBT = B // 128    # batch tiles
KB = IF // 128    # contraction blocks for main matmul
F32 = mybir.dt.float32
F32R = mybir.dt.float32r


def _path_matrices():
    """TmA/TmB [63, 64]: -1.0 where leaf's path takes node as left/right child.

    Mirrors reference._leaf_probs: at level d (n_par = 2^d parents, gate
    columns start..start+n_par), child 2j gets factor g[start+j], child 2j+1
    gets (1 - g[start+j]).
    """
    tma = np.zeros((G, L), dtype=np.float32)
    tmb = np.zeros((G, L), dtype=np.float32)
    start = 0
    for d in range(DEPTH):
        n_par = 2 ** d
        for leaf in range(L):
            j = leaf >> (DEPTH - d)          # ancestor among 2^d nodes
            child = leaf >> (DEPTH - d - 1)
            node = start + j
            if child & 1:
                tmb[node, leaf] = -1.0       # right child: factor (1 - g)
            else:
                tma[node, leaf] = -1.0       # left child: factor g
        start += n_par
    return tma, tmb


_NC_CACHE = None


def _build():
    global _NC_CACHE
    if _NC_CACHE is not None:
        return _NC_CACHE
    nc = bacc.Bacc("TRN2", target_bir_lowering=False, debug=False,
                   num_devices=NCORES)

    # ---- DRAM I/O (per-core values supplied via in_maps) ----
    xgaT = nc.dram_tensor("xgaT", [GF + 1, B], F32, kind="ExternalInput").ap()
    gwa = nc.dram_tensor("gwa", [GF + 1, G], F32, kind="ExternalInput").ap()
    tma = nc.dram_tensor("tma", [G, LPC], F32, kind="ExternalInput").ap()
    tmb = nc.dram_tensor("tmb", [G, LPC], F32, kind="ExternalInput").ap()
    xT = nc.dram_tensor("xT", [IF, B], F32R, kind="ExternalInput").ap()
    pwt = nc.dram_tensor("pwt", [LPC, IF, OF], F32R, kind="ExternalInput").ap()
    pbt = nc.dram_tensor("pbt", [LPC, OF], F32, kind="ExternalInput").ap()
    out = nc.dram_tensor("out", [B // NCORES, OF], F32, kind="ExternalOutput").ap()
    partial = nc.dram_tensor("partial", [B, OF], F32).ap()
    rs_out = nc.dram_tensor("rs_out", [B // NCORES, OF], F32).ap()

    with tile.TileContext(nc) as tc:
        with tc.tile_pool(name="const", bufs=1) as cpool, \
             tc.tile_pool(name="wts", bufs=1) as wpool, \
             tc.tile_pool(name="work", bufs=2) as work, \
             tc.tile_pool(name="psy", bufs=4, space="PSUM") as psy, \
             tc.tile_pool(name="pss", bufs=2, space="PSUM") as pss, \
             tc.tile_pool(name="psb", bufs=2, space="PSUM") as psb:

            # ---------- input DMAs ----------
            # gating inputs first (gating compute is the serial prologue)
            xga_t = []
            gwa_t = []
            for k in range(4):
                t = cpool.tile([128, B], F32, tag=f"xga{k}")
                nc.sync.dma_start(t[:], xgaT[k * 128:(k + 1) * 128, :])
                xga_t.append(t)
                t2 = cpool.tile([128, G], F32, tag=f"gwa{k}")
                nc.sync.dma_start(t2[:], gwa[k * 128:(k + 1) * 128, :])
                gwa_t.append(t2)
            xga_last = cpool.tile([1, B], F32, tag="xga4")
            nc.sync.dma_start(xga_last[:], xgaT[GF:GF + 1, :])
            xga_t.append(xga_last)
            gwa_last = cpool.tile([1, G], F32, tag="gwa4")
            nc.sync.dma_start(gwa_last[:], gwa[GF:GF + 1, :])
            gwa_t.append(gwa_last)
            tma_t = cpool.tile([G, LPC], F32, tag="tma")
            nc.sync.dma_start(tma_t[:], tma[:])
            tmb_t = cpool.tile([G, LPC], F32, tag="tmb")
            nc.sync.dma_start(tmb_t[:], tmb[:])
            pb_t = cpool.tile([LPC, OF], F32, tag="pb")
            nc.sync.dma_start(pb_t[:], pbt[:])

            # main-path inputs
            xt_t = []
            for k in range(KB):
                t = cpool.tile([128, B], F32R, tag=f"xt{k}")
                nc.sync.dma_start(t[:], xT[k * 128:(k + 1) * 128, :])
                xt_t.append(t)
            pw_t = {}
            for j in range(LPC):
                for k in range(KB):
                    t = wpool.tile([128, OF], F32R, tag=f"pw{j}_{k}")
                    nc.sync.dma_start(t[:], pwt[j, k * 128:(k + 1) * 128, :])
                    pw_t[(j, k)] = t

            # ---------- gating network (fp32, exact) ----------
            # zT[g, b] = (x_gating_aug @ gwa).T : psum [63, 512] per half
            # softplus(-z) = ln(1 + exp(-z)); softplus(z) = z + softplus(-z)
            # (only exp/ln are available together in one ACT table)
            spm = cpool.tile([G, B], F32, tag="spm")   # softplus(-z)
            spp = cpool.tile([G, B], F32, tag="spp")   # softplus(+z)
            for h in range(2):
                hs = slice(h * 512, (h + 1) * 512)
                zt_ps = pss.tile([G, 512], F32, tag="small")
                for k in range(5):
                    nc.tensor.matmul(zt_ps[:], gwa_t[k][:],
                                     xga_t[k][:, hs],
                                     start=(k == 0), stop=(k == 4))
                ez = work.tile([G, 512], F32, tag="ez")
                nc.scalar.activation(ez[:], zt_ps[:],
                                     mybir.ActivationFunctionType.Exp,
                                     scale=-1.0)
                nc.scalar.activation(spm[:, hs], ez[:],
                                     mybir.ActivationFunctionType.Ln,
                                     bias=1.0)
                nc.vector.tensor_add(spp[:, hs], zt_ps[:], spm[:, hs])

            # lp[b, l] per batch tile: [128, 8] = exp(spm_sliceT @ TmA + spp_sliceT @ TmB)
            lp_sb = []
            for bt in range(NBT):
                lp_ps = pss.tile([128, LPC], F32, tag="small")
                sl = slice(bt * 128, (bt + 1) * 128)
                nc.tensor.matmul(lp_ps[:], spm[:, sl], tma_t[:],
                                 start=True, stop=False)
                nc.tensor.matmul(lp_ps[:], spp[:, sl], tmb_t[:],
                                 start=False, stop=True)
                t = cpool.tile([128, LPC], F32, tag=f"lp{bt}")
                nc.scalar.activation(t[:], lp_ps[:],
                                     mybir.ActivationFunctionType.Exp)
                lp_sb.append(t)

            # lpT[l, b] (for the bias matmul): [8, 1024]
            lpT = cpool.tile([LPC, B], F32, tag="lpT")
            for h in range(2):
                lpt_ps = pss.tile([LPC, 512], F32, tag="small")
                hs = slice(h * 512, (h + 1) * 512)
                nc.tensor.matmul(lpt_ps[:], tma_t[:], spm[:, hs],
                                 start=True, stop=False)
                nc.tensor.matmul(lpt_ps[:], tmb_t[:], spp[:, hs],
                                 start=False, stop=True)
                nc.scalar.activation(lpT[:, hs], lpt_ps[:],
                                     mybir.ActivationFunctionType.Exp)

            # ---------- main loop ----------
            for bt in range(NBT):
                sl = slice(bt * 128, (bt + 1) * 128)
                # bias: acc starts from sum_l lp[b,l] * pb[o,l]  (fp32 matmul)
                bias_ps = psb.tile([128, OF], F32, tag="bias")
                nc.tensor.matmul(bias_ps[:], lpT[:, sl], pb_t[:],
                                 start=True, stop=True)
                acc = cpool.tile([128, OF], F32, tag=f"acc{bt}")
                nc.scalar.copy(acc[:], bias_ps[:])
                for j in range(LPC):
                    ps = psy.tile([128, OF], F32, tag="psy")
                    for k in range(KB):
                        nc.tensor.matmul(ps[:], xt_t[k][:, sl],
                                         pw_t[(j, k)][:],
                                         start=(k == 0), stop=(k == KB - 1))
                    # acc += lp[:, j] * ps   (per-partition scalar)
                    nc.vector.scalar_tensor_tensor(
                        acc[:], ps[:], lp_sb[bt][:, j:j + 1], acc[:],
                        op0=mybir.AluOpType.mult, op1=mybir.AluOpType.add)
                nc.sync.dma_start(partial[sl, :], acc[:])

            # ---------- cross-core reduction ----------
            nc.gpsimd.collective_compute(
                "ReduceScatter", mybir.AluOpType.add,
                replica_groups=[list(range(NCORES))],
                ins=[partial[:]], outs=[rs_out[:]])
            nc.sync.dma_start(out[:], rs_out[:])

    nc.compile()
    _NC_CACHE = nc
    return nc


def _in_maps(x_gating, x_leaf, gw, gb, pw, pb):
    x_gating = np.asarray(x_gating, dtype=np.float32)
    x_leaf = np.asarray(x_leaf, dtype=np.float32)
    gw = np.asarray(gw, dtype=np.float32)
    gb = np.asarray(gb, dtype=np.float32)
    pw = np.asarray(pw, dtype=np.float32)
    pb = np.asarray(pb, dtype=np.float32)

    xgaT = np.ascontiguousarray(
        np.concatenate([x_gating, np.ones((B, 1), np.float32)], axis=1).T)
    gwa = np.ascontiguousarray(np.concatenate([gw, gb[None, :]], axis=0))
    xT = np.ascontiguousarray(x_leaf.T)
    tma, tmb = _path_matrices()

    maps = []
    for c in range(NCORES):
        lc = slice(c * LPC, (c + 1) * LPC)
        maps.append({
            "xgaT": xgaT,
            "gwa": gwa,
            "tma": np.ascontiguousarray(tma[:, lc]),
            "tmb": np.ascontiguousarray(tmb[:, lc]),
            "xT": xT,
            "pwt": np.ascontiguousarray(pw[:, :, lc].transpose(2, 1, 0)),
            "pbt": np.ascontiguousarray(pb[:, lc].T),
        })
    return maps


def _install_trace_hook():
    """Register the NTFF profile hook that the image's antenv lacks."""
    try:
        import types
        import antenv
        if "antenv.axon_hooks" not in sys.modules:
            mod = types.ModuleType("antenv.axon_hooks")
            mod._hook = None
            mod.set_axon_ntff_profile_hook = (
                lambda h, _m=mod: setattr(_m, "_hook", h))
            mod.get_axon_ntff_profile_hook = lambda _m=mod: _m._hook
            sys.modules["antenv.axon_hooks"] = mod
            antenv.axon_hooks = mod
        import trn_agent_boot.trn_boot as tb
        hook = tb._ntff_profile_via_ctypes('/opt/axon/libaxon_pjrt.so')
        sys.modules["antenv.axon_hooks"].set_axon_ntff_profile_hook(hook)
        import concourse.bass_utils as bu
        bu.upload_artifacts = lambda tmpdir: tmpdir
        return True
    except Exception:
        return False


def kernel(x_gating, x_leaf, gw, gb, pw, pb):
    nc = _build()
    maps = _in_maps(x_gating, x_leaf, gw, gb, pw, pb)
    trace = os.environ.get("HME_TRACE") == "1"
    kwargs = {}
    if trace and _install_trace_hook():
        kwargs["trace"] = True
        td = os.environ.get("HME_TRACE_DIR")
        if td:
            os.makedirs(td, exist_ok=True)
            kwargs["tmpdir"] = td
    res = run_bass_kernel_spmd(nc, maps, core_ids=list(range(NCORES)),
                               **kwargs)
    if trace:
        kernel.last_exec_time_ns = res.exec_time_ns
        kernel.last_profile = res.profile_json
        kernel.last_trace = res.instructions_and_trace
    return np.concatenate([res.results[c]["out"] for c in range(NCORES)],
                          axis=0)
